# revision 1
# baseline (speedup 1.0000x reference)
"""Trainium2 Bass kernel for nn_Meta_67078799229377 (relation-network meta-learner).

Sharding: 8 cores = 4 batch elements x 2 halves of the relation-j axis.
Each core runs the full backbone for its batch element's 6 images, then the
relation network for its 18 (i, j) pairs, fully fused on-chip (the
[s,s,m,m,128] tensor never exists in HBM). Host code only reshapes/shards
inputs and combines 144 scores + 24 per-sample CE terms into the 3 scalar
losses.
"""
import os
import numpy as np
import ml_dtypes

import concourse.bass as bass
import concourse.mybir as mybir
import concourse.tile as tile
from concourse import bacc
from concourse.bass_utils import run_bass_kernel_spmd

F32 = mybir.dt.float32
F32R = mybir.dt.float32r
BF16 = mybir.dt.bfloat16
AF = mybir.ActivationFunctionType
OP = mybir.AluOpType

B, S, D = 4, 6, 8
M = D * D            # 64 spatial positions
C2 = 66              # 64 channels + 2 coord channels
H1 = 128             # g-MLP hidden
CO = 64              # g-MLP out
NCls = 64
N_CORES = 8

# Fraction of hdd-gen ops routed to the scalar engine (rest on vector engine).
ACT_HDD_EVERY = 5    # every 5th q goes to ACT


def _build_nc():
    nc = bacc.Bacc("TRN2", target_bir_lowering=False, debug=False,
                   num_devices=N_CORES)

    din = {}
    def dram_in(name, shape, dtype=F32):
        din[name] = nc.dram_tensor(name, list(shape), dtype, kind="ExternalInput")
        return din[name]

    x_patches = dram_in("patches", [27, S, 1024], BF16)
    x_w1 = dram_in("w1", [27, 32], BF16)
    x_w2 = dram_in("w2", [32, 9 * 48], BF16)
    x_w3 = dram_in("w3", [48, 9 * 64], BF16)
    x_bc1 = dram_in("bc1", [32, 1])
    x_bc2 = dram_in("bc2", [48, 1])
    x_bc3 = dram_in("bc3", [64, 1])
    x_coords = dram_in("coords", [2, S * M], BF16)
    x_wle = dram_in("wle", [65, NCls])
    x_onehot = dram_in("onehot", [S, NCls])
    x_w1a = dram_in("w1a", [C2, H1], BF16)
    x_w1b = dram_in("w1b", [C2, H1], BF16)
    x_bg1 = dram_in("bg1", [H1, 1])
    x_wg2 = dram_in("wg2", [H1, CO], BF16)
    x_bg2 = dram_in("bg2_2", [2 * CO, 1])
    x_wf1e = dram_in("wf1e", [65, 16])
    x_wf2e = dram_in("wf2e", [17, 1])

    out_scores = nc.dram_tensor("scores", [18, 1], F32, kind="ExternalOutput")
    out_cls = nc.dram_tensor("clsv", [S, 1], F32, kind="ExternalOutput")

    with tile.TileContext(nc) as tc:
        with (
            tc.tile_pool(name="const", bufs=1) as cpool,
            tc.tile_pool(name="work", bufs=1) as wpool,
            tc.tile_pool(name="patch", bufs=1) as ppool,
            tc.tile_pool(name="hdd", bufs=2) as hpool,
            tc.tile_pool(name="gscr", bufs=2) as spool,
            tc.tile_pool(name="pconv", bufs=2, space="PSUM") as pc_pool,
            tc.tile_pool(name="pbig", bufs=2, space="PSUM") as pb_pool,
            tc.tile_pool(name="psmall", bufs=2, space="PSUM") as ps_pool,
        ):
            # ---- constants to SBUF ----
            def c_tile(src, shape, dtype=F32):
                t = cpool.tile(list(shape), dtype, tag=src.name)
                nc.sync.dma_start(out=t[:], in_=src[:])
                return t

            w1_sb = c_tile(x_w1, [27, 32], BF16)
            w2_sb = c_tile(x_w2, [32, 9 * 48], BF16)
            w3_sb = c_tile(x_w3, [48, 9 * 64], BF16)
            bc1_sb = c_tile(x_bc1, [32, 1])
            bc2_sb = c_tile(x_bc2, [48, 1])
            bc3_sb = c_tile(x_bc3, [64, 1])
            wle_sb = c_tile(x_wle, [65, NCls])
            onehot_sb = c_tile(x_onehot, [S, NCls])
            w1a_sb = c_tile(x_w1a, [C2, H1], BF16)
            w1b_sb = c_tile(x_w1b, [C2, H1], BF16)
            bg1_sb = c_tile(x_bg1, [H1, 1])
            wg2_sb = c_tile(x_wg2, [H1, CO], BF16)
            bg2_sb = c_tile(x_bg2, [2 * CO, 1])
            wf1e_sb = c_tile(x_wf1e, [65, 16])
            wf2e_sb = c_tile(x_wf2e, [17, 1])

            patches_sb = ppool.tile([27, S, 1024], BF16)
            nc.sync.dma_start(out=patches_sb[:], in_=x_patches[:])

            featc = wpool.tile([C2, S * M], BF16)
            nc.sync.dma_start(out=featc[64:66, :], in_=x_coords[:])

            def r32(ap):
                return ap

            _stages = ["c1", "c2", "c3", "cls", "uv", "rel", "full"]
            _stop = os.environ.get("KSTOP", "full")
            def _do(stage):
                return _stages.index(stage) <= _stages.index(_stop)


            # ---- conv1: [27]->[32], 64x64 -> 32x32 (stride 2, im2col'd) ----
            c1sb = wpool.tile([32, S, 33, 33], BF16)
            for img in range(S):
                # zero the padding strip (row 32 and col 32)
                nc.gpsimd.memset(c1sb[:, img, 32, :], 0.0)
                nc.gpsimd.memset(c1sb[:, img, 0:32, 32], 0.0)
            for img in range(S):
                for h in range(2):
                    ps1 = pc_pool.tile([32, 16, 32], F32, tag="psc")
                    nc.tensor.matmul(
                        ps1[:].rearrange("p a b -> p (a b)"),
                        r32(w1_sb[:]),
                        r32(patches_sb[:, img, h * 512:(h + 1) * 512]),
                        start=True, stop=True)
                    # relu(x + bc1) -> padded layout; alternate engines
                    out_ap = c1sb[:, img, h * 16:(h + 1) * 16, 0:32]
                    if img % 2 == 0:
                        nc.scalar.activation(out_ap, ps1[:], AF.Relu, bias=bc1_sb[:])
                    else:
                        nc.vector.tensor_scalar(out_ap, ps1[:], bc1_sb[:], 0.0,
                                                op0=OP.add, op1=OP.max)

            if _do("c2"):
                # ---- conv2: [32]->[48], 32x32 -> 16x16 ----
                c2sb = wpool.tile([48, S, 17, 17], BF16)
                for img in range(S):
                    nc.gpsimd.memset(c2sb[:, img, 16, :], 0.0)
                    nc.gpsimd.memset(c2sb[:, img, 0:16, 16], 0.0)
                for ip in range(3):      # image pairs
                    ps2 = pc_pool.tile([48, 2, 16, 16], F32, tag="psc")
                    for k, (dy, dx) in enumerate((dy, dx) for dy in range(3) for dx in range(3)):
                        nc.tensor.matmul(
                            ps2[:],
                            r32(w2_sb[:, k * 48:(k + 1) * 48]),
                            r32(c1sb[:, 2 * ip:2 * ip + 2, dy:dy + 31:2, dx:dx + 31:2]),
                            start=(k == 0), stop=(k == 8))
                    out_ap = c2sb[:, 2 * ip:2 * ip + 2, 0:16, 0:16]
                    if ip % 2 == 0:
                        nc.scalar.activation(out_ap, ps2[:], AF.Relu, bias=bc2_sb[:])
                    else:
                        nc.vector.tensor_scalar(out_ap, ps2[:], bc2_sb[:], 0.0,
                                                op0=OP.add, op1=OP.max)

            if _do("c3"):
                # ---- conv3: [48]->[64], 16x16 -> 8x8 ----
                ps3 = ps_pool.tile([64, S, D, D], F32, tag="sm")
                for k, (dy, dx) in enumerate((dy, dx) for dy in range(3) for dx in range(3)):
                    nc.tensor.matmul(
                        ps3[:],
                        r32(w3_sb[:, k * 64:(k + 1) * 64]),
                        r32(c2sb[:, :, dy:dy + 15:2, dx:dx + 15:2]),
                        start=(k == 0), stop=(k == 8))
                nc.scalar.activation(featc[0:64, :].rearrange("p (i m) -> p i m", m=M),
                                     ps3[:].rearrange("p i a b -> p i (a b)"),
                                     AF.Relu, bias=bc3_sb[:])

            if _do("cls"):
                # ---- cls head ----
                fme = wpool.tile([65, S], F32)
                nc.gpsimd.memset(fme[:], 1.0)
                nc.vector.tensor_reduce(
                    fme[0:64, :], featc[0:64, :].rearrange("p (i m) -> p i m", m=M),
                    axis=mybir.AxisListType.X, op=OP.add)
                psl = ps_pool.tile([S, NCls], F32, tag="sm")
                nc.tensor.matmul(psl[:], r32(fme[:]), r32(wle_sb[:]), start=True, stop=True)
                mx = wpool.tile([S, 1], F32)
                nc.vector.tensor_reduce(mx[:], psl[:], axis=mybir.AxisListType.X, op=OP.max)
                shifted = wpool.tile([S, NCls], F32)
                nc.vector.tensor_scalar(shifted[:], psl[:], mx[:], None, op0=OP.subtract)
                escr = wpool.tile([S, NCls], F32)
                se = wpool.tile([S, 1], F32)
                nc.scalar.activation(escr[:], shifted[:], AF.Exp, accum_out=se[:])
                lse = wpool.tile([S, 1], F32)
                nc.scalar.activation(lse[:], se[:], AF.Ln)
                selscr = wpool.tile([S, NCls], F32)
                sel = wpool.tile([S, 1], F32)
                nc.vector.tensor_tensor(selscr[:], shifted[:], onehot_sb[:], op=OP.mult)
                nc.vector.tensor_reduce(sel[:], selscr[:], axis=mybir.AxisListType.X, op=OP.add)
                clsv = wpool.tile([S, 1], F32)
                nc.vector.tensor_tensor(clsv[:], lse[:], sel[:], op=OP.subtract)
                nc.sync.dma_start(out=out_cls[:], in_=clsv[:])

            if _do("uv"):
                # ---- u / v ----
                psu = ps_pool.tile([H1, S * M], F32, tag="sm")
                psv = ps_pool.tile([H1, S * M], F32, tag="sm")
                nc.tensor.matmul(psu[:], r32(w1a_sb[:]), r32(featc[:]), start=True, stop=True)
                nc.tensor.matmul(psv[:], r32(w1b_sb[:]), r32(featc[:]), start=True, stop=True)
                u_f32 = wpool.tile([H1, S * M], F32)
                v_bf = wpool.tile([H1, S * M], BF16)
                v_f32 = wpool.tile([H1, S * M], F32)
                nc.scalar.activation(u_f32[:], psu[:], AF.Copy)
                nc.vector.tensor_scalar(v_bf[:], psv[:], bg1_sb[:], None, op0=OP.add)
                nc.vector.tensor_scalar(v_f32[:], psv[:], bg1_sb[:], None, op0=OP.add)

            if _do("rel"):
                # ---- relation stage ----
                xf_cols = wpool.tile([2 * CO, 36], F32)
                nc.gpsimd.memset(xf_cols[:], 0.0)
                max_units = int(os.environ.get("KUNITS", "6"))
                unit_no = 0
                for jl in range(3):
                    for qh in range(2):
                        unit_no += 1
                        if unit_no > max_units:
                            continue
                        hdd = hpool.tile([H1, 32, S * M], BF16, tag="hdd")
                        for ql in range(32):
                            q = qh * 32 + ql
                            ucol = u_f32[:, jl * M + q: jl * M + q + 1]
                            if ql % ACT_HDD_EVERY == ACT_HDD_EVERY - 1:
                                nc.scalar.activation(hdd[:, ql, :], v_f32[:],
                                                     AF.Relu, bias=ucol)
                            else:
                                nc.vector.tensor_scalar(hdd[:, ql, :], v_bf[:],
                                                        ucol, 0.0,
                                                        op0=OP.add, op1=OP.max)
                        for duo in range(3):
                            iA, iB = 2 * duo, 2 * duo + 1
                            for gh in range(2):
                                ps = pb_pool.tile([2 * CO, 1024], F32, tag="gps")
                                for q2 in range(2):
                                    qg = gh * 2 + q2
                                    nc.tensor.matmul(
                                        ps[0:CO, q2 * 512:(q2 + 1) * 512],
                                        wg2_sb[:],
                                        hdd[:, qg * 8:(qg + 1) * 8, iA * M:(iA + 1) * M],
                                        start=True, stop=True)
                                    nc.tensor.matmul(
                                        ps[CO:2 * CO, q2 * 512:(q2 + 1) * 512],
                                        wg2_sb[:],
                                        hdd[:, qg * 8:(qg + 1) * 8, iB * M:(iB + 1) * M],
                                        start=True, stop=True,
                                        tile_position=(0, 64))
                                ucol_i = (((jl * 2 + qh) * 3 + duo) * 2) + gh
                                gscr = spool.tile([2 * CO, 1024], BF16, tag="gscr")
                                nc.scalar.activation(gscr[:], ps[:], AF.Relu,
                                                     bias=bg2_sb[:],
                                                     accum_out=xf_cols[:, ucol_i:ucol_i + 1])

            if _do("rel"):
                # ---- score head ----
                # sum the two gh-halves, then the two qh-halves
                xf18 = wpool.tile([2 * CO, 18], F32)
                nc.vector.tensor_tensor(
                    xf18[:],
                    xf_cols[:].rearrange("p (a g) -> p a g", g=2)[:, :, 0],
                    xf_cols[:].rearrange("p (a g) -> p a g", g=2)[:, :, 1],
                    op=OP.add)
                # xf_pair[:, jl*3+d] = xf18[:, jl*6+d] + xf18[:, jl*6+3+d]
                xf_pair = wpool.tile([2 * CO, 3, 3], F32)
                nc.vector.tensor_tensor(
                    xf_pair[:],
                    xf18[:].rearrange("p (a b) -> p a b", a=6)[:, 0:6:2, :],
                    xf18[:].rearrange("p (a b) -> p a b", a=6)[:, 1:6:2, :],
                    op=OP.add)
                xf_ext = wpool.tile([65, 18], F32)
                nc.gpsimd.memset(xf_ext[:], 1.0)
                # even local-pair columns <- partitions 0:64 (i = 2d)
                nc.vector.tensor_copy(
                    xf_ext[0:64, :].rearrange("p (a b) -> p a b", a=3)[:, :, 0:6:2],
                    xf_pair[0:64, :, :])
                # odd local-pair columns <- partitions 64:128 (i = 2d+1), needs DMA
                nc.sync.dma_start(
                    out=xf_ext[0:64, :].rearrange("p (a b) -> p a b", a=3)[:, :, 1:6:2],
                    in_=xf_pair[64:128, :, :])
                psh1 = ps_pool.tile([16, 18], F32, tag="sm")
                nc.tensor.matmul(psh1[:], r32(wf1e_sb[:]), r32(xf_ext[:]),
                                 start=True, stop=True)
                h1e = wpool.tile([17, 18], F32)
                nc.gpsimd.memset(h1e[:], 1.0)
                nc.scalar.activation(h1e[0:16, :], psh1[:], AF.Relu)
                psh2 = ps_pool.tile([18, 1], F32, tag="sm")
                nc.tensor.matmul(psh2[:], r32(h1e[:]), r32(wf2e_sb[:]),
                                 start=True, stop=True)
                en = wpool.tile([18, 1], F32)
                nc.scalar.activation(en[:], psh2[:], AF.Exp, scale=-1.0)
                ep1 = wpool.tile([18, 1], F32)
                nc.vector.tensor_scalar(ep1[:], en[:], 1.0, None, op0=OP.add)
                sc = wpool.tile([18, 1], F32)
                nc.vector.reciprocal(sc[:], ep1[:])
                nc.sync.dma_start(out=out_scores[:], in_=sc[:])

            if not _do("cls"):
                d2 = wpool.tile([S, 1], F32, tag="dummy2")
                nc.gpsimd.memset(d2[:], 0.0)
                nc.sync.dma_start(out=out_cls[:], in_=d2[:])
            if not _do("rel"):
                d1 = wpool.tile([18, 1], F32, tag="dummy1")
                nc.gpsimd.memset(d1[:], 0.0)
                nc.sync.dma_start(out=out_scores[:], in_=d1[:])
    nc.compile()
    return nc


_NC_CACHE = None


def _get_nc():
    global _NC_CACHE
    if _NC_CACHE is None:
        _NC_CACHE = _build_nc()
    return _NC_CACHE


def _host_prep(inputs):
    ins = {k: np.asarray(v) for k, v in inputs.items()}
    x = np.concatenate([ins['support_x'], ins['query_x']], axis=1)
    lab = np.concatenate([ins['support_y'], ins['query_y']], axis=1)

    xpad = np.pad(x.astype(np.float32), ((0, 0), (0, 0), (0, 0), (0, 1), (0, 1)))
    win = np.lib.stride_tricks.sliding_window_view(xpad, (3, 3), axis=(3, 4))
    win = win[:, :, :, ::2, ::2]
    patches = win.transpose(0, 2, 5, 6, 1, 3, 4).reshape(B, 27, S, 1024)
    patches = np.ascontiguousarray(patches, np.float32)

    f32 = np.float32
    bf16 = ml_dtypes.bfloat16
    w1 = np.ascontiguousarray(ins['k1'].reshape(32, 27).T, f32).astype(bf16)
    w2 = np.ascontiguousarray(ins['k2'].transpose(1, 2, 3, 0).reshape(32, 9 * 48), f32).astype(bf16)
    w3 = np.ascontiguousarray(ins['k3'].transpose(1, 2, 3, 0).reshape(48, 9 * 64), f32).astype(bf16)

    ii = np.arange(D, dtype=f32) / D
    coord = np.stack([np.broadcast_to(ii[:, None], (D, D)),
                      np.broadcast_to(ii[None, :], (D, D))]).reshape(2, M)
    coords = np.ascontiguousarray(np.tile(coord, (1, S)), f32).astype(bf16)

    onehots = np.zeros((B, S, NCls), f32)
    for b in range(B):
        onehots[b, np.arange(S), lab[b]] = 1.0

    Wg1 = ins['Wg1'].astype(f32)
    common = dict(
        w1=w1, w2=w2, w3=w3,
        bc1=np.ascontiguousarray(ins['bc1'].reshape(32, 1), f32),
        bc2=np.ascontiguousarray(ins['bc2'].reshape(48, 1), f32),
        bc3=np.ascontiguousarray(ins['bc3'].reshape(64, 1), f32),
        coords=coords,
        wle=np.ascontiguousarray(
            np.vstack([ins['Wlog'].astype(f32) / M, ins['blog'][None, :].astype(f32)])),
        w1a=np.ascontiguousarray(Wg1[:C2]).astype(bf16),
        w1b=np.ascontiguousarray(Wg1[C2:]).astype(bf16),
        bg1=np.ascontiguousarray(ins['bg1'].reshape(H1, 1), f32),
        wg2=np.ascontiguousarray(ins['Wg2'], f32).astype(ml_dtypes.bfloat16),
        bg2_2=np.ascontiguousarray(np.tile(ins['bg2'].astype(f32), 2).reshape(2 * CO, 1)),
        wf1e=np.ascontiguousarray(
            np.vstack([ins['Wf1'].astype(f32), ins['bf1'][None, :].astype(f32)])),
        wf2e=np.ascontiguousarray(
            np.vstack([ins['Wf2'].astype(f32), ins['bf2'].reshape(1, 1).astype(f32)])),
    )
    in_maps = []
    for core in range(N_CORES):
        b, half = core // 2, core % 2
        # odd cores see images in rotated order so the program's local
        # j in {0,1,2} maps to global j in {3,4,5}
        perm = (0, 1, 2, 3, 4, 5) if half == 0 else (3, 4, 5, 0, 1, 2)
        m = dict(common)
        m['patches'] = np.ascontiguousarray(patches[b][:, perm, :]).astype(ml_dtypes.bfloat16)
        m['onehot'] = np.ascontiguousarray(onehots[b][list(perm)])
        in_maps.append(m)
    return in_maps, lab


def _host_post(results, lab):
    P = np.zeros((B, S, S), np.float32)
    cls_terms = np.zeros((B, S), np.float32)
    for core in range(N_CORES):
        b, half = core // 2, core % 2
        perm = (0, 1, 2, 3, 4, 5) if half == 0 else (3, 4, 5, 0, 1, 2)
        sc = results[core]["scores"].reshape(18)
        for jl in range(3):
            for i in range(S):
                P[b, perm[i], perm[jl]] = sc[jl * 6 + i]
        if half == 0:
            cls_terms[b] = results[core]["clsv"].reshape(S)
    cls_loss = np.float32(cls_terms.mean())
    y = (lab[:, :, None] == lab[:, None, :]).astype(np.float32)
    Pt = P.transpose(0, 2, 1)
    sym, anti = np.float32(0.5) * (P + Pt), np.float32(0.5) * (P - Pt)
    sym_n = np.sqrt((sym ** 2).sum(axis=(1, 2)))
    anti_n = np.sqrt((anti ** 2).sum(axis=(1, 2)))
    sym_loss = np.float32(((sym_n - anti_n) / (sym_n + anti_n)).mean())
    euc_loss = np.float32(((P - y) ** 2).mean())
    rn_loss = np.float32(euc_loss - np.float32(0.1) * sym_loss)
    return np.float32(cls_loss), np.float32(rn_loss), np.float32(sym_loss)


def run_spmd(inputs, trace=False, **kwargs):
    nc = _get_nc()
    in_maps, lab = _host_prep(inputs)
    res = run_bass_kernel_spmd(nc, in_maps, list(range(N_CORES)),
                               trace=trace, **kwargs)
    return _host_post(res.results, lab), res


def kernel(**inputs):
    out, _ = run_spmd(inputs)
    return out



# revision 2
# speedup vs baseline: 1.4068x; 1.4068x over previous
"""Trainium2 Bass kernel for nn_Meta_67078799229377 (relation-network meta-learner).

Sharding: 8 cores = 4 batch elements x 2 halves of the relation-j axis.
Each core runs the full backbone for its batch element's 6 images, then the
relation g-MLP for its 18 (i, j) pairs fully fused on-chip.  The device only
produces (a) per-image channel sums `fme` for the cls head and (b) the
(q,p)-summed relation features `xf`; the tiny f/cls MLP heads and loss
reductions run on the host in f64.

Engine plan (measured rates):
  - hdd = relu(v + u_q): DVE tensor_scalar at 4x-mode (~230ns / [128,384]),
    a configurable few on ACT.
  - g matmuls: PE pairs at tile_position (0,0)/(0,64) which overlap in the
    array (2 cols/cycle effective); 2048-col PSUM tiles, one per (unit, duo).
  - gscr relu+bias+sum: ACT activation with accum_out ([128,2048] ~2.15us).
  - Pool(gpsimd): memsets only (tensor ops are ~50x slow on HW).
  - PE warmup matmuls + early ACT table trigger overlap the input DMA.
"""
import os
import numpy as np
import ml_dtypes

import concourse.bass as bass
import concourse.mybir as mybir
import concourse.tile as tile
from concourse import bacc
from concourse.bass_utils import run_bass_kernel_spmd

F32 = mybir.dt.float32
BF16 = mybir.dt.bfloat16
AF = mybir.ActivationFunctionType
OP = mybir.AluOpType

B, S, D = 4, 6, 8
M = D * D            # 64 spatial positions
C2 = 66              # 64 channels + 2 coord channels
H1 = 128             # g-MLP hidden
CO = 64              # g-MLP out
NCls = 64
N_CORES = 8

# bf16 const blob column layout
CB_W1 = 0            # [27, 32]
CB_W2 = 32           # [32, 432]
CB_W3 = 464          # [48, 576]
CB_W1A = 1040        # [66, 128]
CB_W1B = 1168        # [66, 128]
CB_WG2 = 1296        # [128, 64]
CB_COORD = 1360      # [2, 384]
CB_N = 1744

# f32 const blob column layout: bc1, bc2, bc3, bg1, bg2(x2)
CF_N = 5

KWARM = int(os.environ.get("KWARM", "10"))
KH_ACT = int(os.environ.get("KH_ACT", "1"))   # hdd instrs per unit on ACT
KG_V = int(os.environ.get("KG_V", "0"))       # gscr instrs per unit on DVE


def _build_nc():
    nc = bacc.Bacc("TRN2", target_bir_lowering=False, debug=False,
                   num_devices=N_CORES)

    x_patches = nc.dram_tensor("patches", [27, S, 1024], BF16,
                               kind="ExternalInput")
    x_cb = nc.dram_tensor("cb", [128, CB_N], BF16, kind="ExternalInput")
    x_cf = nc.dram_tensor("cf", [128, CF_N], F32, kind="ExternalInput")

    out_fme = nc.dram_tensor("fme", [64, S], F32, kind="ExternalOutput")
    out_xfa = nc.dram_tensor("xfa", [128, 18], F32, kind="ExternalOutput")
    out_xfd = nc.dram_tensor("xfd", [128, 18], F32, kind="ExternalOutput")

    with tile.TileContext(nc) as tc:
        with (
            tc.tile_pool(name="const", bufs=1) as cpool,
            tc.tile_pool(name="work", bufs=1) as wpool,
            tc.tile_pool(name="patch", bufs=1) as ppool,
            tc.tile_pool(name="hdd", bufs=2) as hpool,
            tc.tile_pool(name="gscr", bufs=2) as spool,
        ):
            # ---- scratch + warmup (no input deps: runs during DMA) ----
            wsrc = cpool.tile([128, 512], BF16, tag="wsrc")
            nc.gpsimd.memset(wsrc[:], 0.0)
            ttrig = wpool.tile([128, 2], F32, tag="ttrig")

            cb = cpool.tile([128, CB_N], BF16, tag="cb")
            cf = cpool.tile([128, CF_N], F32, tag="cf")
            nc.sync.dma_start(out=cb[:], in_=x_cb[:])
            nc.sync.dma_start(out=cf[:], in_=x_cf[:])

            patches_sb = ppool.tile([27, S, 1024], BF16)
            for c in range(3):
                nc.sync.dma_start(out=patches_sb[:, 2 * c:2 * c + 2, :],
                                  in_=x_patches[:, 2 * c:2 * c + 2, :])

            # trigger the ACT function-table load early (relu set)
            nc.scalar.activation(ttrig[:], wsrc[:, 0:2], AF.Relu)

            w1 = cb[0:27, CB_W1:CB_W1 + 32]
            w2 = cb[0:32, CB_W2:CB_W2 + 432]
            w3 = cb[0:48, CB_W3:CB_W3 + 576]
            w1a = cb[0:C2, CB_W1A:CB_W1A + H1]
            w1b = cb[0:C2, CB_W1B:CB_W1B + H1]
            wg2 = cb[:, CB_WG2:CB_WG2 + CO]
            bc1 = cf[0:32, 0:1]
            bc2 = cf[0:48, 1:2]
            bc3 = cf[0:64, 2:3]
            bg1 = cf[:, 3:4]
            bg2 = cf[:, 4:5]

            featc = wpool.tile([C2, S * M], BF16)
            nc.vector.tensor_copy(featc[64:66, :], cb[0:2, CB_COORD:CB_COORD + 384])

            xf_a = wpool.tile([128, 18], F32, tag="xfa")
            xf_d = wpool.tile([128, 18], F32, tag="xfd")
            nc.gpsimd.memset(xf_a[:], 0.0)
            nc.gpsimd.memset(xf_d[:], 0.0)

            zb2048 = cpool.tile([128, 2048], BF16, tag="zb")
            if KG_V > 0:
                nc.gpsimd.memset(zb2048[:], 0.0)

            c1sb = wpool.tile([32, S, 33, 33], BF16)
            c2sb = wpool.tile([48, S, 17, 17], BF16)
            for img in range(S):
                nc.gpsimd.memset(c1sb[:, img, 32, :], 0.0)
                nc.gpsimd.memset(c1sb[:, img, 0:32, 32], 0.0)
                nc.gpsimd.memset(c2sb[:, img, 16, :], 0.0)
                nc.gpsimd.memset(c2sb[:, img, 0:16, 16], 0.0)

            with tc.tile_pool(name="pwarm", bufs=1, space="PSUM") as pw_pool:
                psw = pw_pool.tile([128, 512], F32, tag="warm")
                for r in range(KWARM):
                    nc.tensor.matmul(psw[0:64, :], wsrc[:, 0:64], wsrc[:],
                                     start=True, stop=True, tile_position=(0, 0))
                    nc.tensor.matmul(psw[64:128, :], wsrc[:, 0:64], wsrc[:],
                                     start=True, stop=True, tile_position=(0, 64))

            with (
                tc.tile_pool(name="pconv", bufs=2, space="PSUM") as pc_pool,
                tc.tile_pool(name="psmall", bufs=2, space="PSUM") as ps_pool,
            ):
                # ---- conv1: [27]->[32], im2col'd, 64x64 -> 32x32 ----
                for img in range(S):
                    for h in range(2):
                        ps1 = pc_pool.tile([32, 16, 32], F32, tag="psc")
                        nc.tensor.matmul(
                            ps1[:].rearrange("p a b -> p (a b)"),
                            w1,
                            patches_sb[:, img, h * 512:(h + 1) * 512],
                            start=True, stop=True)
                        out_ap = c1sb[:, img, h * 16:(h + 1) * 16, 0:32]
                        if (img * 2 + h) % 2 == 0:
                            nc.scalar.activation(out_ap, ps1[:], AF.Relu, bias=bc1)
                        else:
                            nc.vector.tensor_scalar(out_ap, ps1[:], bc1, 0.0,
                                                    op0=OP.add, op1=OP.max)

                # ---- conv2: [32]->[48], 32x32 -> 16x16 ----
                for ip in range(3):
                    ps2 = pc_pool.tile([48, 2, 16, 16], F32, tag="psc")
                    for k, (dy, dx) in enumerate(
                            (dy, dx) for dy in range(3) for dx in range(3)):
                        nc.tensor.matmul(
                            ps2[:],
                            w2[:, k * 48:(k + 1) * 48],
                            c1sb[:, 2 * ip:2 * ip + 2, dy:dy + 31:2, dx:dx + 31:2],
                            start=(k == 0), stop=(k == 8))
                    out_ap = c2sb[:, 2 * ip:2 * ip + 2, 0:16, 0:16]
                    if ip % 2 == 0:
                        nc.scalar.activation(out_ap, ps2[:], AF.Relu, bias=bc2)
                    else:
                        nc.vector.tensor_scalar(out_ap, ps2[:], bc2, 0.0,
                                                op0=OP.add, op1=OP.max)

                # ---- conv3: [48]->[64], 16x16 -> 8x8 ----
                ps3 = ps_pool.tile([64, S, D, D], F32, tag="sm")
                for k, (dy, dx) in enumerate(
                        (dy, dx) for dy in range(3) for dx in range(3)):
                    nc.tensor.matmul(
                        ps3[:],
                        w3[:, k * 64:(k + 1) * 64],
                        c2sb[:, :, dy:dy + 15:2, dx:dx + 15:2],
                        start=(k == 0), stop=(k == 8))
                nc.scalar.activation(
                    featc[0:64, :].rearrange("p (i m) -> p i m", m=M),
                    ps3[:].rearrange("p i a b -> p i (a b)"),
                    AF.Relu, bias=bc3)

                # ---- cls: per-image channel sums, rest on host ----
                fme = wpool.tile([64, S], F32)
                nc.vector.tensor_reduce(
                    fme[:], featc[0:64, :].rearrange("p (i m) -> p i m", m=M),
                    axis=mybir.AxisListType.X, op=OP.add)
                nc.sync.dma_start(out=out_fme[:], in_=fme[:])

                # ---- u / v ----
                psu = ps_pool.tile([H1, S * M], F32, tag="sm")
                psv = ps_pool.tile([H1, S * M], F32, tag="sm")
                nc.tensor.matmul(psu[:], w1a, featc[:], start=True, stop=True)
                nc.tensor.matmul(psv[:], w1b, featc[:], start=True, stop=True)
                u_f32 = wpool.tile([H1, S * M], F32)
                v_bf = wpool.tile([H1, S * M], BF16)
                nc.scalar.activation(u_f32[:], psu[:], AF.Copy)
                nc.vector.tensor_scalar(v_bf[:], psv[:], bg1, None, op0=OP.add)

            # ---- relation stage ----
            hdd_act = set(range(32 - KH_ACT, 32))
            with tc.tile_pool(name="pbig", bufs=2, space="PSUM") as pb_pool:
                for jl in range(3):
                    for qh in range(2):
                        unit = jl * 2 + qh
                        hdd = hpool.tile([H1, 32, S * M], BF16, tag="hdd")
                        for ql in range(32):
                            q = qh * 32 + ql
                            ucol = u_f32[:, jl * M + q: jl * M + q + 1]
                            if ql in hdd_act:
                                nc.scalar.activation(hdd[:, ql, :], v_bf[:],
                                                     AF.Relu, bias=ucol)
                            else:
                                nc.vector.tensor_scalar(hdd[:, ql, :], v_bf[:],
                                                        ucol, 0.0,
                                                        op0=OP.add, op1=OP.max)
                        for duo in range(3):
                            iA, iB = 2 * duo, 2 * duo + 1
                            ps = pb_pool.tile([128, 2048], F32, tag="gps")
                            for qg in range(4):
                                nc.tensor.matmul(
                                    ps[0:CO, qg * 512:(qg + 1) * 512],
                                    wg2,
                                    hdd[:, qg * 8:(qg + 1) * 8, iA * M:(iA + 1) * M],
                                    start=True, stop=True,
                                    tile_position=(0, 0))
                                nc.tensor.matmul(
                                    ps[CO:2 * CO, qg * 512:(qg + 1) * 512],
                                    wg2,
                                    hdd[:, qg * 8:(qg + 1) * 8, iB * M:(iB + 1) * M],
                                    start=True, stop=True,
                                    tile_position=(0, 64))
                            col = unit * 3 + duo
                            gscr = spool.tile([128, 2048], BF16, tag="gscr")
                            if duo < KG_V:
                                nc.vector.scalar_tensor_tensor(
                                    gscr[:], ps[:], bg2, zb2048[:],
                                    op0=OP.add, op1=OP.max,
                                    accum_out=xf_d[:, col:col + 1])
                            else:
                                nc.scalar.activation(
                                    gscr[:], ps[:], AF.Relu, bias=bg2,
                                    accum_out=xf_a[:, col:col + 1])

            nc.sync.dma_start(out=out_xfa[:], in_=xf_a[:])
            nc.sync.dma_start(out=out_xfd[:], in_=xf_d[:])
    nc.compile()
    return nc


_NC_CACHE = None


def _get_nc():
    global _NC_CACHE
    if _NC_CACHE is None:
        _NC_CACHE = _build_nc()
    return _NC_CACHE


def _host_prep(inputs):
    ins = {k: np.asarray(v) for k, v in inputs.items()}
    x = np.concatenate([ins['support_x'], ins['query_x']], axis=1)
    lab = np.concatenate([ins['support_y'], ins['query_y']], axis=1)

    xpad = np.pad(x.astype(np.float32), ((0, 0), (0, 0), (0, 0), (0, 1), (0, 1)))
    win = np.lib.stride_tricks.sliding_window_view(xpad, (3, 3), axis=(3, 4))
    win = win[:, :, :, ::2, ::2]
    patches = win.transpose(0, 2, 5, 6, 1, 3, 4).reshape(B, 27, S, 1024)
    patches = np.ascontiguousarray(patches, np.float32)

    f32 = np.float32
    bf16 = ml_dtypes.bfloat16

    cb = np.zeros((128, CB_N), f32)
    cb[0:27, CB_W1:CB_W1 + 32] = ins['k1'].reshape(32, 27).T
    cb[0:32, CB_W2:CB_W2 + 432] = ins['k2'].transpose(1, 2, 3, 0).reshape(32, 432)
    cb[0:48, CB_W3:CB_W3 + 576] = ins['k3'].transpose(1, 2, 3, 0).reshape(48, 576)
    Wg1 = ins['Wg1'].astype(f32)
    cb[0:C2, CB_W1A:CB_W1A + H1] = Wg1[:C2]
    cb[0:C2, CB_W1B:CB_W1B + H1] = Wg1[C2:]
    cb[:, CB_WG2:CB_WG2 + CO] = ins['Wg2']
    ii = np.arange(D, dtype=f32) / D
    coord = np.stack([np.broadcast_to(ii[:, None], (D, D)),
                      np.broadcast_to(ii[None, :], (D, D))]).reshape(2, M)
    cb[0:2, CB_COORD:CB_COORD + 384] = np.tile(coord, (1, S))
    cb = cb.astype(bf16)

    cf = np.zeros((128, CF_N), f32)
    cf[0:32, 0] = ins['bc1']
    cf[0:48, 1] = ins['bc2']
    cf[0:64, 2] = ins['bc3']
    cf[:, 3] = ins['bg1']
    cf[:, 4] = np.tile(ins['bg2'].astype(f32), 2)

    in_maps = []
    for core in range(N_CORES):
        b, half = core // 2, core % 2
        perm = (0, 1, 2, 3, 4, 5) if half == 0 else (3, 4, 5, 0, 1, 2)
        m = dict(cb=cb, cf=cf)
        m['patches'] = np.ascontiguousarray(patches[b][:, perm, :]).astype(bf16)
        in_maps.append(m)
    return in_maps, lab, ins


def _host_post(results, lab, ins):
    f64 = np.float64
    Wf1 = ins['Wf1'].astype(f64)
    bf1 = ins['bf1'].astype(f64)
    Wf2 = ins['Wf2'].astype(f64)
    bf2 = ins['bf2'].astype(f64)
    Wlog = ins['Wlog'].astype(f64)
    blog = ins['blog'].astype(f64)

    P = np.zeros((B, S, S), f64)
    cls_terms = np.zeros((B, S), f64)
    for core in range(N_CORES):
        b, half = core // 2, core % 2
        perm = (0, 1, 2, 3, 4, 5) if half == 0 else (3, 4, 5, 0, 1, 2)
        xf = (results[core]["xfa"].astype(f64)
              + results[core]["xfd"].astype(f64))       # [128, 18]
        xf9 = xf.reshape(128, 3, 2, 3).sum(axis=2)      # (jl, duo)
        for jl in range(3):
            for duo in range(3):
                for par in range(2):
                    i = 2 * duo + par
                    vec = xf9[par * 64:(par + 1) * 64, jl, duo]
                    h = np.maximum(vec @ Wf1 + bf1, 0.0)
                    z = h @ Wf2 + bf2
                    P[b, perm[i], perm[jl]] = 1.0 / (1.0 + np.exp(-z[0]))
        if half == 0:
            fme = results[core]["fme"].astype(f64)      # [64, S] channel sums
            logits = (fme.T / M) @ Wlog + blog          # [S, NCls]
            mx = logits.max(axis=1)
            lse = mx + np.log(np.exp(logits - mx[:, None]).sum(axis=1))
            cls_terms[b] = lse - logits[np.arange(S), lab[b]]

    cls_loss = cls_terms.mean()
    y = (lab[:, :, None] == lab[:, None, :]).astype(f64)
    Pt = P.transpose(0, 2, 1)
    sym, anti = 0.5 * (P + Pt), 0.5 * (P - Pt)
    sym_n = np.sqrt((sym ** 2).sum(axis=(1, 2)))
    anti_n = np.sqrt((anti ** 2).sum(axis=(1, 2)))
    sym_loss = ((sym_n - anti_n) / (sym_n + anti_n)).mean()
    euc_loss = ((P - y) ** 2).mean()
    rn_loss = euc_loss - 0.1 * sym_loss
    return np.float32(cls_loss), np.float32(rn_loss), np.float32(sym_loss)


def run_spmd(inputs, trace=False, **kwargs):
    nc = _get_nc()
    in_maps, lab, ins = _host_prep(inputs)
    res = run_bass_kernel_spmd(nc, in_maps, list(range(N_CORES)),
                               trace=trace, **kwargs)
    return _host_post(res.results, lab, ins), res


def kernel(**inputs):
    out, _ = run_spmd(inputs)
    return out
